# revision 1
# baseline (speedup 1.0000x reference)
"""Fused multi-head attention for Trainium2, SPMD over 8 NeuronCores — v2.

Sharding: core c handles batch c//2, query rows [(c%2)*1024, ...) — same as
v1 (data parallel over batch x query-length; K/V projections recomputed by
the two cores sharing a batch).

Differences vs v1:
- Inputs arrive host-pre-transposed ([128, 4, rows]) and stream in as
  plain DMAs: the XBAR DMA-transposes serialized ~8us apiece on the
  HWDGE ring and dominated the old startup.
- Scores run as two K=64 row-tiled matmuls (heads 2m / 2m+1 at
  partitions 0:64 / 64:128); qT/kT use the packed layout, no zero pads.
- V-projection and all A@V matmuls trail their exp by a rolling lag
  (cross-pair carry), so the PE never bulk-drains while ACT starves and
  exp->A@V dependencies are always met without pipeline restarts.
- K/Q projections for pair p+1 are injected block-by-block into pair
  p's stream.
- softmax z-normalization: DVE reciprocal straight from PSUM row 0 +
  gpsimd partition-broadcast of the reciprocal.
- Wo ships bf16 from the host.

PSUM budget (8 banks): sc ring 2 x [128,2,512] f32 (4) + av_e/av_o
[65,1024] f32 (4).
"""

import numpy as np

B, L, S, D, H, E = 4, 2048, 2048, 512, 8, 64
LC = L // 2
N_CORES = 8
SC = S // 128        # 16 s-chunks
QC = LC // 512       # 2 q-chunks of 512

# (pair, chunk, qc) combos whose exp runs on DVE (Schraudolph) instead of ACT.
DVE_EXP = set()  # PE-bound: keep exp exact on ACT

# A@V emission lag in qc-iterations: pair 0 waits out the xv transpose,
# later pairs just need exp->matmul slack + av-free slack at pair starts.
LAG0 = 10
LAGN = 4

# Schraudolph constants for bf16-via-int16: I = round(x * SCHRA_A + SCHRA_B),
# bitcast int16 -> bf16 gives ~exp(x/8) (scale 1/sqrt(E) folded into A).
SCHRA_A = 128.0 / (8.0 * np.log(2.0))
SCHRA_B = 127.0 * 128.0 - 7.5  # DVE f32->int cast rounds to nearest

_cached = None


def _build_bass():
    import concourse.bacc as bacc
    import concourse.mybir as mybir
    from concourse.tile import TileContext

    f32 = mybir.dt.float32
    bf16 = mybir.dt.bfloat16
    i16 = mybir.dt.int16
    AF = mybir.ActivationFunctionType
    ALU = mybir.AluOpType

    nc = bacc.Bacc("TRN2", target_bir_lowering=False, debug=False,
                   num_devices=N_CORES)

    # Inputs pre-TRANSPOSED on the host to [128, 4, rows] (d-chunk on
    # partitions): XBAR DMA-transposes serialize ~8us apiece on the HWDGE
    # ring (full-completion semaphore chain), so the transpose is done as
    # host-side layout prep and the load becomes a plain wide DMA.
    xq = nc.dram_tensor("xq", [128, 4, LC], bf16, kind="ExternalInput")
    xk = nc.dram_tensor("xk", [128, 4, S], bf16, kind="ExternalInput")
    xv = nc.dram_tensor("xv", [128, 4, S], bf16, kind="ExternalInput")
    wq = nc.dram_tensor("wq", [D, D], bf16, kind="ExternalInput")
    wk = nc.dram_tensor("wk", [D, D], bf16, kind="ExternalInput")
    wv = nc.dram_tensor("wv", [D, 8 * 65], bf16, kind="ExternalInput")
    wo = nc.dram_tensor("wo", [2 * D, D], bf16, kind="ExternalInput")
    bq = nc.dram_tensor("bq", [1, D], f32, kind="ExternalInput")
    bk = nc.dram_tensor("bk", [1, D], f32, kind="ExternalInput")
    bv = nc.dram_tensor("bv", [1, 8 * 65], f32, kind="ExternalInput")
    bo = nc.dram_tensor("bo", [1, D], f32, kind="ExternalInput")
    y = nc.dram_tensor("y", [LC, D], f32, kind="ExternalOutput")

    import contextlib
    with TileContext(nc) as tc, contextlib.ExitStack() as ctx:
        persist = ctx.enter_context(tc.tile_pool(name="persist", bufs=1))

        # K/Q weights first (small, needed first) on the gpsimd (SWDGE)
        # ring.  Each DMA-transpose FENCES against all in-flight DMAs on
        # HW, so the remaining weights (V/O path, needed late) are emitted
        # only after the last transpose.
        wk_sb = persist.tile([128, 4, 512], bf16)
        wq_sb = persist.tile([128, 4, 512], bf16)
        wv_sb = persist.tile([128, 4, 520], bf16)
        wo_sb = persist.tile([128, 8, 512], bf16)
        bqT = persist.tile([128, 4], f32)
        bkT = persist.tile([128, 4], f32)
        bv_bc = persist.tile([128, 520], f32)
        bo_bc = persist.tile([128, 512], f32)
        nc.gpsimd.dma_start(
            out=wk_sb, in_=wk[:, :].rearrange("(c p) d -> p c d", p=128))
        nc.gpsimd.dma_start(
            out=bkT, in_=bk[0:1, :].rearrange("o (m p) -> (o p) m", p=128))
        nc.gpsimd.dma_start(
            out=wq_sb, in_=wq[:, :].rearrange("(c p) d -> p c d", p=128))
        nc.gpsimd.dma_start(
            out=bqT, in_=bq[0:1, :].rearrange("o (m p) -> (o p) m", p=128))

        # Input transposes: [s, d-chunk] -> [d-chunk partitions, s].
        # All on ONE ring (sync): concurrent XBAR streams from two HWDGE
        # rings corrupt each other on HW.  Order = need order: xk (K-proj
        # first), xq, then xv.
        xt = ctx.enter_context(tc.tile_pool(name="xt", bufs=1))
        xkT = xt.tile([128, 4, S], bf16)
        xqT = xt.tile([128, 4, LC], bf16)
        xvT = xt.tile([128, 4, S], bf16)
        # Need-order on ONE ring: concurrent loads share HBM bandwidth and
        # delay xkT (the first-needed tensor) by ~10us.
        nc.sync.dma_start(out=xkT, in_=xk[:, :, :])
        nc.sync.dma_start(out=xqT, in_=xq[:, :, :])
        nc.sync.dma_start(out=xvT, in_=xv[:, :, :])

        # V/O-path weights: emitted after the last transpose (see above).
        nc.gpsimd.dma_start(
            out=wv_sb, in_=wv[:, :].rearrange("(c p) d -> p c d", p=128))
        nc.gpsimd.dma_start(out=bv_bc, in_=bv[0:1, :].broadcast_to((128, 520)))
        nc.gpsimd.dma_start(
            out=wo_sb, in_=wo[:, :].rearrange("(h p) d -> p h d", p=128))
        nc.gpsimd.dma_start(out=bo_bc, in_=bo[0:1, :].broadcast_to((128, 512)))

        # Long-lived attention operands (packed head-pair layout, no pads).
        attn = ctx.enter_context(tc.tile_pool(name="attn", bufs=1))
        qT = attn.tile([128, 4, LC], bf16)     # pair m: head 2m rows 0:64, 2m+1 rows 64:128
        kT = attn.tile([128, 4, S], bf16)
        vaug = attn.tile([128, SC, 8 * 65], bf16)  # per s-chunk: 8x [1 | V_h]
        oT = attn.tile([128, 8, LC], bf16)     # per head: row0=junk, 1:65 O^T, 65:128 zero
        # Rows 65:128 of each head must be zero (K=128 out-proj contraction);
        # row 64 is covered by the memset first, then overwritten by the
        # normalize muls.  Partition base must be 32-aligned, hence 64.
        nc.gpsimd.memset(oT[64:128, :, :], 0.0)

        with tc.tile_pool(name="scp", bufs=2, space="PSUM") as scp, \
             tc.tile_pool(name="avp", bufs=1, space="PSUM") as avp, \
             tc.tile_pool(name="pp", bufs=LAG0 + 3) as pp, \
             tc.tile_pool(name="zrp", bufs=2) as zrp:

            def proj_k_n(m, n):
                # kT columns [n*1024, (n+1)*1024) for head-pair m.
                ps = scp.tile([128, 1024], f32, tag="sc", name=f"psk_{m}_{n}")
                for k in range(4):
                    for half in range(2):
                        nc.tensor.matmul(
                            ps[:, half * 512:(half + 1) * 512],
                            wk_sb[:, k, m * 128:(m + 1) * 128],
                            xkT[:, k, (2 * n + half) * 512:(2 * n + half + 1) * 512],
                            start=(k == 0), stop=(k == 3))
                nc.vector.tensor_add(
                    kT[:, m, 2 * n * 512:(2 * n + 2) * 512],
                    ps,
                    bkT[:, m:m + 1].to_broadcast((128, 1024)))

            def proj_q_h(m, half):
                ps = scp.tile([128, 1024], f32, tag="sc", name=f"psq_{m}_{half}")
                for k in range(4):
                    nc.tensor.matmul(
                        ps[:, 0:512],
                        wq_sb[:, k, m * 128:(m + 1) * 128],
                        xqT[:, k, half * 512:(half + 1) * 512],
                        start=(k == 0), stop=(k == 3))
                nc.vector.tensor_add(
                    qT[:, m, half * 512:(half + 1) * 512],
                    ps[:, 0:512],
                    bqT[:, m:m + 1].to_broadcast((128, 512)))

            def proj_v(i):
                ps = scp.tile([128, 1024], f32, tag="sc", name=f"psv_{i}")
                for k in range(4):
                    for half in range(2):
                        nc.tensor.matmul(
                            ps[:, half * 512:half * 512 + 260],
                            xvT[:, k, i * 128:(i + 1) * 128],
                            wv_sb[:, k, half * 260:(half + 1) * 260],
                            start=(k == 0), stop=(k == 3))
                nc.vector.tensor_add(
                    vaug[:, i, :].rearrange("p (a b) -> p a b", a=2),
                    ps.rearrange("p (a b) -> p a b", a=2)[:, :, 0:260],
                    bv_bc[:, :].rearrange("p (a b) -> p a b", a=2))

            # K/Q projections for pair p+1 are injected one n/half-block at a
            # time at the END of chosen iterations of pair p, so the ~5us of
            # projection matmuls never stall the exp stream.
            def injections(p, i):
                if p == 3:
                    return
                sched = {10: 0, 12: 1} if p == 0 else {6: 0, 8: 1}
                qsched = {13: 0, 15: 1} if p == 0 else {11: 0, 12: 1}
                if i in sched:
                    proj_k_n(p + 1, sched[i])
                if i in qsched:
                    proj_q_h(p + 1, qsched[i])

            # Both K-blocks first: proj_q stalls on the xqT load (arrives
            # ~5us after xkT) and, PE being in-order, would block the second
            # K-block behind it.  Scores chunks 0-7 only need K-block 0.
            proj_k_n(0, 0)
            proj_k_n(0, 1)
            proj_q_h(0, 0)
            proj_q_h(0, 1)

            # Global rolling A@V deferral with CROSS-PAIR carry: each A@V
            # pair is emitted LAG qc-iters after its exp, and a pair's
            # leftover backlog drains during the next pair's score/exp
            # stream (the PE would otherwise bulk-drain it at the pair end
            # while ACT starves).  The softmax-normalize for a pair is
            # emitted right after its last A@V drains.
            pending = []
            emitted_v = set()

            def emit_av(p, i, qc, pt, av_e, av_o):
                if p == 0 and i not in emitted_v:
                    emitted_v.add(i)
                    proj_v(i)
                he, ho = 2 * p, 2 * p + 1
                nc.tensor.matmul(
                    av_e[0:65, qc * 512:(qc + 1) * 512],
                    vaug[:, i, he * 65:(he + 1) * 65],
                    pt[:, 0:512],
                    start=(i == 0), stop=(i == SC - 1))
                nc.tensor.matmul(
                    av_o[0:65, qc * 512:(qc + 1) * 512],
                    vaug[:, i, ho * 65:(ho + 1) * 65],
                    pt[:, 512:1024],
                    start=(i == 0), stop=(i == SC - 1))
                if i == SC - 1 and qc == QC - 1:
                    normalize(p, av_e, av_o)

            def normalize(p, av_e, av_o):
                for h, av in ((2 * p, av_e), (2 * p + 1, av_o)):
                    zinv = zrp.tile([1, 1024], f32, tag="zinv", name=f"zi_{h}")
                    nc.vector.reciprocal_approx_fast(
                        out=zinv[0:1, :], in_=av[0:1, :])
                    bcinv = zrp.tile([65, 1024], f32, tag="bcinv",
                                     name=f"bc_{h}")
                    nc.gpsimd.partition_broadcast(bcinv, zinv[0:1, :])
                    for qc in range(QC):
                        # Row 0 computes z * (1/z) = 1; the matching Wo row
                        # is zero, so it never reaches the output.
                        nc.vector.tensor_mul(
                            oT[0:65, h, qc * 512:(qc + 1) * 512],
                            av[0:65, qc * 512:(qc + 1) * 512],
                            bcinv[0:65, qc * 512:(qc + 1) * 512])

            for p in range(4):
                av_e = avp.tile([65, 1024], f32, tag="av_e", name=f"av_{2 * p}")
                av_o = avp.tile([65, 1024], f32, tag="av_o",
                                name=f"av_{2 * p + 1}")
                for i in range(SC):
                    for qc in range(QC):
                        sc = scp.tile([128, 1024], f32, tag="sc",
                                      name=f"sc_{p}_{i}_{qc}")
                        nc.tensor.matmul(
                            sc[:, 0:512],
                            kT[0:64, p, i * 128:(i + 1) * 128],
                            qT[0:64, p, qc * 512:(qc + 1) * 512],
                            start=True, stop=True)
                        nc.tensor.matmul(
                            sc[:, 512:1024],
                            kT[64:128, p, i * 128:(i + 1) * 128],
                            qT[64:128, p, qc * 512:(qc + 1) * 512],
                            start=True, stop=True)
                        pt = pp.tile([128, 1024], bf16, tag="p",
                                     name=f"p_{p}_{i}_{qc}")
                        if (p, i, qc) in DVE_EXP:
                            nc.vector.tensor_scalar(
                                out=pt.bitcast(i16), in0=sc,
                                scalar1=float(SCHRA_A), scalar2=float(SCHRA_B),
                                op0=ALU.mult, op1=ALU.add)
                        else:
                            nc.scalar.activation(out=pt, in_=sc, func=AF.Exp,
                                                 scale=float(1.0 / np.sqrt(E)))
                        pending.append((p, i, qc, pt, av_e, av_o))
                        if p == 0:
                            # Ramp the lag down 1-per-iter after the V path
                            # is up, so the backlog drains smoothly.
                            t = 2 * i + qc
                            lag = max(LAGN, LAG0 - max(0, t - 23))
                        else:
                            lag = LAGN
                        while len(pending) > lag:
                            emit_av(*pending.pop(0))
                    injections(p, i)
            while pending:
                emit_av(*pending.pop(0))

            # ---- Output projection: Y = O @ Wo + bo, inside the scp scope
            # so yp accumulators reuse the freed scores-PSUM ring.  Heads
            # 0-5 are final once pair 2 normalizes, so their matmuls run
            # while pair 3's softmax-normalize is still in flight; only the
            # h=6,7 matmuls wait on the last normalize.
            with tc.tile_pool(name="ysb", bufs=3) as ysb:
                for lc in range(LC // 128):
                    ypt = scp.tile([128, 1024], f32, tag="sc", name=f"yp_{lc}")
                    yp = ypt[:, 0:512]
                    for h in range(8):
                        nc.tensor.matmul(
                            yp, oT[:, h, lc * 128:(lc + 1) * 128],
                            wo_sb[:, h, :],
                            start=(h == 0), stop=(h == 7))
                    ysb_t = ysb.tile([128, 512], f32, tag="ysb")
                    nc.vector.tensor_add(ysb_t, yp, bo_bc)
                    nc.sync.dma_start(
                        out=y[lc * 128:(lc + 1) * 128, :], in_=ysb_t)

    nc.compile()
    return nc


def _get_compiled():
    global _cached
    if _cached is None:
        _cached = _build_bass()
    return _cached


def make_in_maps(queries, keys, values, Wq, bq, Wk, bk, Wv, bv, Wo, bo):
    import ml_dtypes
    bf16 = ml_dtypes.bfloat16
    f = np.ascontiguousarray

    # Augment Wv/bv with a ones output column per head: the extra column of
    # the A@V matmul accumulates the softmax denominator z.
    wv_aug = np.zeros((D, 8 * 65), dtype=np.float32)
    bv_aug = np.zeros((1, 8 * 65), dtype=np.float32)
    wv_np = np.asarray(Wv, dtype=np.float32)
    bv_np = np.asarray(bv, dtype=np.float32).reshape(D)
    for h in range(8):
        wv_aug[:, h * 65 + 1:h * 65 + 65] = wv_np[:, h * 64:(h + 1) * 64]
        bv_aug[0, h * 65 + 1:h * 65 + 65] = bv_np[h * 64:(h + 1) * 64]
        bv_aug[0, h * 65] = 1.0
    wv_aug = f(wv_aug.astype(bf16))
    bv_aug = f(bv_aug)
    wo_np = np.asarray(Wo, dtype=np.float32)
    wo_pad = np.zeros((2 * D, D), dtype=np.float32)
    for h in range(8):
        wo_pad[h * 128 + 1:h * 128 + 65, :] = wo_np[h * 64:(h + 1) * 64, :]
    wo_pad = f(wo_pad.astype(bf16))
    queries = np.asarray(queries)

    def chunk4(x):
        # [rows, 512] -> transposed [128, 4, rows] contiguous bf16
        xb = np.asarray(x, dtype=np.float32).astype(bf16)
        return f(xb.reshape(-1, 4, 128).transpose(2, 1, 0))

    in_maps = []
    for c in range(N_CORES):
        b, half = c // 2, c % 2
        in_maps.append({
            "xq": chunk4(queries[b, half * LC:(half + 1) * LC, :]),
            "xk": chunk4(np.asarray(keys)[b]),
            "xv": chunk4(np.asarray(values)[b]),
            "wq": f(np.asarray(Wq, dtype=np.float32).astype(bf16)),
            "wk": f(np.asarray(Wk, dtype=np.float32).astype(bf16)),
            "wv": wv_aug,
            "wo": wo_pad,
            "bq": f(np.asarray(bq).reshape(1, D), dtype=np.float32),
            "bk": f(np.asarray(bk).reshape(1, D), dtype=np.float32),
            "bv": bv_aug,
            "bo": f(np.asarray(bo).reshape(1, D), dtype=np.float32),
        })
    return in_maps


def gather_out(results):
    out = np.empty((B, L, D), dtype=np.float32)
    for c in range(N_CORES):
        b, half = c // 2, c % 2
        out[b, half * LC:(half + 1) * LC, :] = results[c]["y"]
    return out


def kernel(queries, keys, values, Wq, bq, Wk, bk, Wv, bv, Wo, bo):
    from concourse.bass_utils import run_bass_kernel_spmd

    nc = _get_compiled()
    in_maps = make_in_maps(queries, keys, values, Wq, bq, Wk, bk, Wv, bv, Wo, bo)
    res = run_bass_kernel_spmd(nc, in_maps, core_ids=list(range(N_CORES)))
    return gather_out(res.results)

